# revision 77
# baseline (speedup 1.0000x reference)
"""Trainium2 Bass kernel for per-sample modulated/demodulated 3x3 conv.

Problem: x (8,512,32,32), s (8,512), w (512,512,3,3) ->
  wm[b,o,i,ky,kx] = w * (s[b,i]+1); demod by rsqrt(sum wm^2 + eps) per (b,o);
  y[b] = conv2d_same(x[b], wm[b]).

1D Winograd F(2,3) along H: y = A^T [ (G_h w) . (B_h^T x') ] with the kx
taps handled directly via shifted-window PSUM accumulation (like a plain
conv), modulation folded into x (x' = x*(1+s)) and demodulation folded
into the output (y *= den[o], den = rsqrt(sum_i (1+s_i)^2 wsq[i,o]+eps)).
The h-transform of the weights (G w along ky -> 4 positions) and wsq are
weight-only preprocessing done host-side; weights ship bf16. PE work is
2/3 of a direct conv; the input transform is 4 contiguous-stride DVE ops
per (cin chunk, sample) (stride-2 only on the middle dim, which DVE runs
at full rate -- inner-stride-2 patterns measured ~2x slower).

Sharding: 4 batch-groups (2 samples each) x 2 cout-halves = 8 cores; each
core loads half the transformed weights (3 MiB bf16) and its 2 samples.

Per-core schedule: two PE waves (sample 0, then sample 1), 8 PSUM banks
per wave (2 cout blocks x 4 h-positions), each bank accumulating 12
matmuls (4 cin chunks x 3 kx shifts) of ~512 bf16 columns. DVE makes
V = B^T x' per chunk (sample 0 first) so wave A starts after one chunk's
4 transform ops. Output transform A^T (z0 = M0+M1+M2 even rows,
z1 = M1-M2-M3 odd rows) splits PSUM reads one-per-op (HW limit): ACT
drains M1/M2 to SBUF, DVE does the adds, ACT applies den and scatters
into even/odd row pairs of the bf16 y tile.
"""

import sys

if "/opt/trn_rl_repo" not in sys.path:
    sys.path.insert(0, "/opt/trn_rl_repo")

import numpy as np
import ml_dtypes

B = 8
CIN = 512
COUT = 512
H = 32
W = 32
NCH = CIN // 128   # cin chunks
PGRID = 4          # batch groups
QGRID = 2          # cout halves
SPC = B // PGRID   # samples per core
OHALF = COUT // QGRID
OB = OHALF // 128  # cout blocks per core
HP = H + 2
EPS = 1e-8
BF = ml_dtypes.bfloat16

_compiled_nc = None


def _build():
    import concourse.tile as tile
    from concourse import bacc, mybir

    F32 = mybir.dt.float32
    BF16 = mybir.dt.bfloat16

    nc = bacc.Bacc("TRN2", target_bir_lowering=False, debug=False, num_devices=B)
    # x ships unpadded: 32x32 bf16 = 2048B partition lines split into clean
    # DMA packets (a 34x34 padded layout gives 2312B lines whose 264B
    # remainder packets halve effective DMA bandwidth)
    xp_d = nc.dram_tensor("xp", [SPC, NCH, 128, H, W], BF16, kind="ExternalInput").ap()
    s1_d = nc.dram_tensor("s1p", [128, NCH, SPC], F32, kind="ExternalInput").ap()
    q_d = nc.dram_tensor("qp", [128, NCH, SPC], BF16, kind="ExternalInput").ap()
    # kx-major so the first conv group needs only the kx0 slab of chunk 0
    wt_d = nc.dram_tensor("wt", [NCH, 128, 3, 4, OHALF], BF16, kind="ExternalInput").ap()
    wsq_d = nc.dram_tensor("wsq", [128, NCH, OHALF], BF16, kind="ExternalInput").ap()
    # y ships as separate even/odd row planes (z0/z1); host interleaves
    y_d = nc.dram_tensor("y", [SPC, OB, 2, 128, 16 * W], BF16, kind="ExternalOutput").ap()

    with tile.TileContext(nc) as tc:
        with (
            tc.tile_pool(name="wpool", bufs=1) as wpool,
            tc.tile_pool(name="xstage", bufs=4) as xstage,
            tc.tile_pool(name="xmp", bufs=1) as xmp,
            tc.tile_pool(name="vp", bufs=1) as vp,
            tc.tile_pool(name="up", bufs=6) as up,
            tc.tile_pool(name="zp", bufs=4) as zp,
            tc.tile_pool(name="yp", bufs=1) as yp,
            tc.tile_pool(name="misc", bufs=1) as misc,
            tc.tile_pool(name="psum", bufs=8, space="PSUM") as psum,
        ):
            wt_sb = [
                wpool.tile([128, 3, 4, OHALF], BF16, name=f"wt{c}", tag=f"wt{c}")
                for c in range(NCH)
            ]
            # xm: h-padded only (rows 0/33 zero); the kx-trimmed matmuls
            # never read w-pad columns, so V carries none
            xm = [
                [xmp.tile([128, HP, W], BF16, name=f"xm{sm}_{c}", tag=f"xm{sm}_{c}")
                 for c in range(NCH)]
                for sm in range(SPC)
            ]
            # V[sm][c][a]: h-transformed input, [128, 16 th, 32 w]
            V = [
                [[vp.tile([128, 16, W], BF16, name=f"v{sm}_{c}_{a}", tag=f"v{sm}_{c}_{a}")
                  for a in range(4)]
                 for c in range(NCH)]
                for sm in range(SPC)
            ]

            s1 = misc.tile([128, NCH, SPC], F32, name="s1", tag="s1")
            q = misc.tile([128, NCH, SPC], BF16, name="q", tag="q")
            wsq_sb = misc.tile([128, NCH, OHALF], BF16, name="wsq_sb", tag="wsq_sb")
            den_s = misc.tile([128, OB, SPC], F32, name="den_s", tag="den_s")
            den = misc.tile([128, OB, SPC], F32, name="den", tag="den")
            dneg = misc.tile([128, OB, SPC], F32, name="dneg", tag="dneg")
            eps_t = misc.tile([128, 1], F32, name="eps_t", tag="eps_t")
            junk = misc.tile([128, 512], BF16, name="junk", tag="junk")
            scr = misc.tile([128, 1], F32, name="scr", tag="scr")

            nc.vector.memset(eps_t, EPS)
            nc.vector.memset(junk, 0.0)
            # prewarm the ACT copy + sqrt tables during the DMA shadow so the
            # real mods/sqrt don't pay the 1.5us ACT_TABLE_LOAD on-path
            nc.scalar.mul(scr, eps_t, 2.0)
            nc.scalar.activation(scr, eps_t,
                                 mybir.ActivationFunctionType.Sqrt, bias=eps_t)

            # --- PE warmup while the first DMAs land (HAM clock ramp):
            # many small matmuls keep PE continuously busy cheaply
            warm = psum.tile([128, 512], F32, name="warm", tag="acc")
            for _ in range(28):
                nc.tensor.matmul(warm[:, 0:256], lhsT=junk[:, 0:128],
                                 rhs=junk[:, 0:256], start=True, stop=True)

            # --- DMAs. A queue feeds only ~4 DMA engines (~80 GB/s), so the
            # critical transfers are spread across queues: x(0,0) ships in
            # partition halves on gpsimd+scalar concurrently (keeps 2048B
            # lines -- DMA cost is per partition-line packet), wt-c0 right
            # behind s1 on sync.
            xs = {}
            for sm in range(SPC):
                for c in range(NCH):
                    xs[(sm, c)] = xstage.tile([128, H, W], BF16,
                                              name=f"xs{sm}_{c}", tag="xstage",
                                              bufs=4)
            # gpsimd's SWDGE has ~3.3us completion latency vs ~1us on the
            # sync/scalar HWDGE queues -- everything start-critical goes on
            # the HWDGE queues, interleaved so demod inputs (q/wsq), the
            # first x chunk (partition-halved across both queues) and the c0
            # weights (h-positions split across both queues) all land early.
            nc.sync.dma_start(out=xs[(0, 0)][0:64], in_=xp_d[0, 0, 0:64])
            nc.scalar.dma_start(out=xs[(0, 0)][64:128], in_=xp_d[0, 0, 64:128])
            nc.sync.dma_start(out=s1, in_=s1_d)
            nc.sync.dma_start(out=q, in_=q_d)
            nc.scalar.dma_start(out=wsq_sb, in_=wsq_d)
            nc.sync.dma_start(out=wt_sb[0][:, 0, :, :], in_=wt_d[0, :, 0, :, :])
            nc.sync.dma_start(out=wt_sb[0][:, 1, :, :], in_=wt_d[0, :, 1, :, :])
            nc.sync.dma_start(out=wt_sb[0][:, 2, :, :], in_=wt_d[0, :, 2, :, :])
            nc.gpsimd.dma_start(out=xs[(0, 1)], in_=xp_d[0, 1])
            nc.scalar.dma_start(out=xs[(0, 2)], in_=xp_d[0, 2])
            nc.scalar.dma_start(out=xs[(0, 3)], in_=xp_d[0, 3])
            for c in range(NCH):
                nc.gpsimd.dma_start(out=xs[(1, c)], in_=xp_d[1, c])
            for c in range(1, NCH):
                nc.sync.dma_start(out=wt_sb[c], in_=wt_d[c])
            # zero the h-pad rows on DVE: it is idle during the x DMA wait,
            # and on gpsimd these would queue behind 9 DMA descriptor gens,
            # gating the first V transform
            for sm in range(SPC):
                for c in range(NCH):
                    nc.vector.memset(xm[sm][c][:, 0, :], 0.0)
                    nc.vector.memset(xm[sm][c][:, HP - 1, :], 0.0)

            # --- demod matvec: dsum[:,ob,:] = sum_c wsq[c,ob-block]^T (1+s)^2
            # Matmuls are emitted mid-wave (after the first conv group) so
            # the wsq transfer is off conv's critical path; den_s sits at the
            # head of ACT's queue. The M tile recycling dsum's slot has its
            # first write pushed a full group later (see wave()).
            dsum = psum.tile([128, OB, SPC], F32, name="dsum", tag="acc")
            for ob in range(OB):
                for c in range(NCH):
                    nc.tensor.matmul(
                        dsum[:, ob, :],
                        lhsT=wsq_sb[:, c, ob * 128:(ob + 1) * 128],
                        rhs=q[:, c, :],
                        start=(c == 0), stop=(c == NCH - 1),
                    )
            for ob in range(OB):
                nc.scalar.activation(den_s[:, ob, :], dsum[:, ob, :],
                                     mybir.ActivationFunctionType.Sqrt,
                                     bias=eps_t)

            # --- modulation + input transform: the first chunk's mod runs on
            # DVE (lowest latency for wave A's start); the rest on ACT so mod
            # and transform pipeline across engines. V transforms on DVE.
            for sm in range(SPC):
                for c in range(NCH):
                    if sm == 0 and c == 0:
                        nc.vector.tensor_scalar_mul(xm[sm][c][:, 1:H + 1, :],
                                                    xs[(sm, c)],
                                                    s1[:, c, sm:sm + 1])
                    else:
                        nc.scalar.mul(xm[sm][c][:, 1:H + 1, :], xs[(sm, c)],
                                      s1[:, c, sm:sm + 1])
                    xr = xm[sm][c].rearrange("p (a b) w -> p a b w", b=2)
                    ev0, ev1 = xr[:, 0:16, 0, :], xr[:, 1:17, 0, :]
                    od0, od1 = xr[:, 0:16, 1, :], xr[:, 1:17, 1, :]
                    Vc = V[sm][c]
                    nc.vector.tensor_sub(Vc[0], ev0, ev1)
                    nc.vector.tensor_add(Vc[1], od0, ev1)
                    nc.vector.tensor_sub(Vc[2], ev1, od0)
                    nc.vector.tensor_sub(Vc[3], od0, od1)
                if sm == 0:
                    nc.vector.reciprocal(den, den_s)
                    nc.vector.tensor_scalar_mul(dneg, den, -1.0)

            # --- conv waves: per sample, 8 banks = (2 ob x 4 a), each
            # accumulating 12 matmuls (4 c x 3 kx shifted windows). The
            # zero-pad cols of V contribute nothing, so kx=0 skips out col 0
            # and kx=2 skips out col 31 (PSUM has_written covers first hits).
            M = {}

            # V has no w-pad: kx window = [c_lo+kx-1, ...) in V coords
            KXW = {0: (1, 0, 31), 1: (0, 0, 32), 2: (0, 1, 31)}  # out_lo, v_lo, n

            def wave(sm):
                def mm(c, ob, kx, a):
                    o_lo, v_lo, n_c = KXW[kx]
                    key = (sm, ob, a)
                    if key not in M:
                        M[key] = psum.tile([128, 16, W], F32,
                                           name=f"m{sm}_{ob}_{a}", tag="acc")
                    nc.tensor.matmul(
                        M[key][:, :, o_lo:o_lo + n_c],
                        lhsT=wt_sb[c][:, kx, a, ob * 128:(ob + 1) * 128],
                        rhs=V[sm][c][a][:, :, v_lo:v_lo + n_c],
                        start=(c == 0 and kx == 0),
                        stop=(c == NCH - 1 and kx == 2),
                    )

                # kx-outer over both cout blocks: a bank is revisited only
                # every 8 matmuls (4-apart hits a PSUM turnaround gap). The
                # last chunk ends with the kx2 groups per-ob, a-order 1,2,3,0
                # -- the drain chain starts from M1 (ACT d1 copy), so its
                # accumulator should stop first and M0 (needed one op later)
                # last.
                for c in range(NCH):
                    for kx in range(3):
                        for ob in range(OB):
                            if c == NCH - 1 and kx == 2:
                                continue
                            for a in range(4):
                                mm(c, ob, kx, a)
                    if c == NCH - 1:
                        for ob in range(OB):
                            for a in (1, 2, 3, 0):
                                mm(c, ob, 2, a)

            def drain(sm, ob):
                # z0 = (M0+M1+M2)*den -> even rows; z1 = (M1-M2-M3)*den ->
                # odd rows. den folds into the adds via scalar_tensor_tensor
                # (out = (in0*scalar) op in1): ACT pre-scales d1s = M1*den,
                # the z1 path uses -den so no trailing negate/scale is
                # needed. Each op reads at most one PSUM bank. z0/z1 DMA out
                # directly as row-parity planes; the host interleaves rows.
                AL = mybir.AluOpType
                dn = den[:, ob, sm:sm + 1]
                dg = dneg[:, ob, sm:sm + 1]
                # z1-path first: M1/M2/M3 stop before M0 (kx2 a-order 1,2,3,0)
                # so it overlaps the final matmuls; only u0->z0 trails them.
                m = [M[(sm, ob, a)].rearrange("p t w -> p (t w)") for a in range(4)]
                d1s = up.tile([128, 512], F32, name=f"d1_{sm}_{ob}", tag="u", bufs=6)
                nc.scalar.mul(d1s, m[1], dn)
                u1 = up.tile([128, 512], F32, name=f"u1_{sm}_{ob}", tag="u", bufs=6)
                z1 = zp.tile([128, 512], BF16, name=f"z1_{sm}_{ob}", tag="z", bufs=4)
                nc.vector.scalar_tensor_tensor(u1, m[2], dg, d1s, AL.mult, AL.add)
                nc.vector.scalar_tensor_tensor(z1, m[3], dg, u1, AL.mult, AL.add)
                # odd-row plane ships via the scalar queue (idle at the tail)
                # so both planes' DMA descriptors generate concurrently
                nc.scalar.dma_start(out=y_d[sm, ob, 1], in_=z1)
                u0 = up.tile([128, 512], F32, name=f"u0_{sm}_{ob}", tag="u", bufs=6)
                z0 = zp.tile([128, 512], BF16, name=f"z0_{sm}_{ob}", tag="z", bufs=4)
                nc.vector.scalar_tensor_tensor(u0, m[0], dn, d1s, AL.mult, AL.add)
                nc.vector.scalar_tensor_tensor(z0, m[2], dn, u0, AL.mult, AL.add)
                nc.sync.dma_start(out=y_d[sm, ob, 0], in_=z0)

            wave(0)
            drain(0, 0)
            drain(0, 1)
            wave(1)
            drain(1, 0)
            drain(1, 1)

    nc.compile()
    return nc


_G = np.array(
    [[1.0, 0.0, 0.0], [0.5, 0.5, 0.5], [0.5, -0.5, 0.5], [0.0, 0.0, 1.0]],
    np.float32)


def prepare_in_maps(x, s, w):
    """Shard + pack full inputs into per-core in_maps (core = g*QGRID + h)."""
    x = np.asarray(x, dtype=np.float32)
    s = np.asarray(s, dtype=np.float32)
    w = np.asarray(w, dtype=np.float32)

    # h-transformed weights: wt1[kx,a,i,o] = sum_p G[a,p] w[o,i,p,kx]
    wt1 = np.einsum("ap,oipk->kaio", _G, w, optimize=True)
    # -> [cin, kx, a, cout] -> [NCH,128,3,4,COUT]
    wt_l = np.ascontiguousarray(wt1.transpose(2, 0, 1, 3)).reshape(
        NCH, 128, 3, 4, COUT).astype(BF)
    wsq = np.sum(w * w, axis=(2, 3)).T  # [cin, cout]
    wsq_l = np.ascontiguousarray(
        wsq.reshape(NCH, 128, COUT).transpose(1, 0, 2)).astype(BF)

    xp_all = x.astype(BF).reshape(PGRID, SPC, NCH, 128, H, W)
    s1_all = (s + 1.0).reshape(PGRID, SPC, NCH, 128).transpose(0, 3, 2, 1)
    q_all = (s1_all * s1_all).astype(BF)

    in_maps = []
    for g in range(PGRID):
        for h in range(QGRID):
            in_maps.append({
                "xp": np.ascontiguousarray(xp_all[g]),
                "s1p": np.ascontiguousarray(s1_all[g]),
                "qp": np.ascontiguousarray(q_all[g]),
                "wt": np.ascontiguousarray(wt_l[:, :, :, :, h * OHALF:(h + 1) * OHALF]),
                "wsq": np.ascontiguousarray(wsq_l[:, :, h * OHALF:(h + 1) * OHALF]),
            })
    return in_maps


def assemble_output(results):
    y = np.zeros((B, COUT, H, W), np.float32)
    for g in range(PGRID):
        for h in range(QGRID):
            r = results[g * QGRID + h]["y"].astype(np.float32)
            for sm in range(SPC):
                for ob in range(OB):
                    blk = y[g * SPC + sm,
                            h * OHALF + ob * 128:h * OHALF + (ob + 1) * 128]
                    blk[:, 0::2, :] = r[sm, ob, 0].reshape(128, 16, W)
                    blk[:, 1::2, :] = r[sm, ob, 1].reshape(128, 16, W)
    return y


def kernel(x, s, w):
    from concourse.bass_utils import run_bass_kernel_spmd

    global _compiled_nc
    if _compiled_nc is None:
        _compiled_nc = _build()
    nc = _compiled_nc

    in_maps = prepare_in_maps(x, s, w)
    res = run_bass_kernel_spmd(nc, in_maps, list(range(B))).results
    return assemble_output(res)


# revision 78
# speedup vs baseline: 1.0235x; 1.0235x over previous
"""Trainium2 Bass kernel for per-sample modulated/demodulated 3x3 conv.

Problem: x (8,512,32,32), s (8,512), w (512,512,3,3) ->
  wm[b,o,i,ky,kx] = w * (s[b,i]+1); demod by rsqrt(sum wm^2 + eps) per (b,o);
  y[b] = conv2d_same(x[b], wm[b]).

1D Winograd F(2,3) along H: y = A^T [ (G_h w) . (B_h^T x') ] with the kx
taps handled directly via shifted-window PSUM accumulation (like a plain
conv), modulation folded into x (x' = x*(1+s)) and demodulation folded
into the output (y *= den[o], den = rsqrt(sum_i (1+s_i)^2 wsq[i,o]+eps)).
The h-transform of the weights (G w along ky -> 4 positions) and wsq are
weight-only preprocessing done host-side; weights ship bf16. PE work is
2/3 of a direct conv; the input transform is 4 contiguous-stride DVE ops
per (cin chunk, sample) (stride-2 only on the middle dim, which DVE runs
at full rate -- inner-stride-2 patterns measured ~2x slower).

Sharding: 4 batch-groups (2 samples each) x 2 cout-halves = 8 cores; each
core loads half the transformed weights (3 MiB bf16) and its 2 samples.

Per-core schedule: two PE waves (sample 0, then sample 1), 8 PSUM banks
per wave (2 cout blocks x 4 h-positions), each bank accumulating 12
matmuls (4 cin chunks x 3 kx shifts) of ~512 bf16 columns. DVE makes
V = B^T x' per chunk (sample 0 first) so wave A starts after one chunk's
4 transform ops. Output transform A^T (z0 = M0+M1+M2 even rows,
z1 = M1-M2-M3 odd rows) splits PSUM reads one-per-op (HW limit): ACT
drains M1/M2 to SBUF, DVE does the adds, ACT applies den and scatters
into even/odd row pairs of the bf16 y tile.
"""

import sys

if "/opt/trn_rl_repo" not in sys.path:
    sys.path.insert(0, "/opt/trn_rl_repo")

import numpy as np
import ml_dtypes

B = 8
CIN = 512
COUT = 512
H = 32
W = 32
NCH = CIN // 128   # cin chunks
PGRID = 4          # batch groups
QGRID = 2          # cout halves
SPC = B // PGRID   # samples per core
OHALF = COUT // QGRID
OB = OHALF // 128  # cout blocks per core
HP = H + 2
EPS = 1e-8
BF = ml_dtypes.bfloat16

_compiled_nc = None


def _build():
    import concourse.tile as tile
    from concourse import bacc, mybir

    F32 = mybir.dt.float32
    BF16 = mybir.dt.bfloat16

    nc = bacc.Bacc("TRN2", target_bir_lowering=False, debug=False, num_devices=B)
    # x ships unpadded: 32x32 bf16 = 2048B partition lines split into clean
    # DMA packets (a 34x34 padded layout gives 2312B lines whose 264B
    # remainder packets halve effective DMA bandwidth)
    xp_d = nc.dram_tensor("xp", [SPC, NCH, 128, H, W], BF16, kind="ExternalInput").ap()
    s1_d = nc.dram_tensor("s1p", [128, NCH, SPC], F32, kind="ExternalInput").ap()
    q_d = nc.dram_tensor("qp", [128, NCH, SPC], BF16, kind="ExternalInput").ap()
    # kx-major so the first conv group needs only the kx0 slab of chunk 0
    wt_d = nc.dram_tensor("wt", [NCH, 128, 3, 4, OHALF], BF16, kind="ExternalInput").ap()
    wsq_d = nc.dram_tensor("wsq", [128, NCH, OHALF], BF16, kind="ExternalInput").ap()
    # y ships as separate even/odd row planes (z0/z1); host interleaves
    y_d = nc.dram_tensor("y", [SPC, OB, 2, 128, 16 * W], BF16, kind="ExternalOutput").ap()

    with tile.TileContext(nc) as tc:
        with (
            tc.tile_pool(name="wpool", bufs=1) as wpool,
            tc.tile_pool(name="xstage", bufs=4) as xstage,
            tc.tile_pool(name="xmp", bufs=1) as xmp,
            tc.tile_pool(name="vp", bufs=1) as vp,
            tc.tile_pool(name="up", bufs=6) as up,
            tc.tile_pool(name="zp", bufs=4) as zp,
            tc.tile_pool(name="yp", bufs=1) as yp,
            tc.tile_pool(name="misc", bufs=1) as misc,
            tc.tile_pool(name="psum", bufs=8, space="PSUM") as psum,
        ):
            wt_sb = [
                wpool.tile([128, 3, 4, OHALF], BF16, name=f"wt{c}", tag=f"wt{c}")
                for c in range(NCH)
            ]
            # xm: h-padded only (rows 0/33 zero); the kx-trimmed matmuls
            # never read w-pad columns, so V carries none
            xm = [
                [xmp.tile([128, HP, W], BF16, name=f"xm{sm}_{c}", tag=f"xm{sm}_{c}")
                 for c in range(NCH)]
                for sm in range(SPC)
            ]
            # V[sm][c][a]: h-transformed input, [128, 16 th, 32 w]
            V = [
                [[vp.tile([128, 16, W], BF16, name=f"v{sm}_{c}_{a}", tag=f"v{sm}_{c}_{a}")
                  for a in range(4)]
                 for c in range(NCH)]
                for sm in range(SPC)
            ]

            s1 = misc.tile([128, NCH, SPC], F32, name="s1", tag="s1")
            q = misc.tile([128, NCH, SPC], BF16, name="q", tag="q")
            wsq_sb = misc.tile([128, NCH, OHALF], BF16, name="wsq_sb", tag="wsq_sb")
            den_s = misc.tile([128, OB, SPC], F32, name="den_s", tag="den_s")
            den = misc.tile([128, OB, SPC], F32, name="den", tag="den")
            dneg = misc.tile([128, OB, SPC], F32, name="dneg", tag="dneg")
            eps_t = misc.tile([128, 1], F32, name="eps_t", tag="eps_t")
            junk = misc.tile([128, 512], BF16, name="junk", tag="junk")
            scr = misc.tile([128, 1], F32, name="scr", tag="scr")

            nc.vector.memset(eps_t, EPS)
            nc.vector.memset(junk, 0.0)
            # prewarm the ACT copy + sqrt tables during the DMA shadow so the
            # real mods/sqrt don't pay the 1.5us ACT_TABLE_LOAD on-path
            nc.scalar.mul(scr, eps_t, 2.0)
            nc.scalar.activation(scr, eps_t,
                                 mybir.ActivationFunctionType.Sqrt, bias=eps_t)

            # --- PE warmup while the first DMAs land (HAM clock ramp):
            # many small matmuls keep PE continuously busy cheaply
            warm = psum.tile([128, 512], F32, name="warm", tag="acc")
            for _ in range(28):
                nc.tensor.matmul(warm[:, 0:256], lhsT=junk[:, 0:128],
                                 rhs=junk[:, 0:256], start=True, stop=True)

            # --- DMAs. A queue feeds only ~4 DMA engines (~80 GB/s), so the
            # critical transfers are spread across queues: x(0,0) ships in
            # partition halves on gpsimd+scalar concurrently (keeps 2048B
            # lines -- DMA cost is per partition-line packet), wt-c0 right
            # behind s1 on sync.
            xs = {}
            for sm in range(SPC):
                for c in range(NCH):
                    xs[(sm, c)] = xstage.tile([128, H, W], BF16,
                                              name=f"xs{sm}_{c}", tag="xstage",
                                              bufs=4)
            # gpsimd's SWDGE has ~3.3us completion latency vs ~1us on the
            # sync/scalar HWDGE queues -- everything start-critical goes on
            # the HWDGE queues, interleaved so demod inputs (q/wsq), the
            # first x chunk (partition-halved across both queues) and the c0
            # weights (h-positions split across both queues) all land early.
            nc.sync.dma_start(out=xs[(0, 0)][0:64], in_=xp_d[0, 0, 0:64])
            nc.scalar.dma_start(out=xs[(0, 0)][64:128], in_=xp_d[0, 0, 64:128])
            nc.sync.dma_start(out=s1, in_=s1_d)
            nc.sync.dma_start(out=q, in_=q_d)
            nc.scalar.dma_start(out=wsq_sb, in_=wsq_d)
            nc.sync.dma_start(out=wt_sb[0][:, 0, :, :], in_=wt_d[0, :, 0, :, :])
            nc.sync.dma_start(out=wt_sb[0][:, 1, :, :], in_=wt_d[0, :, 1, :, :])
            nc.sync.dma_start(out=wt_sb[0][:, 2, :, :], in_=wt_d[0, :, 2, :, :])
            nc.gpsimd.dma_start(out=xs[(0, 1)], in_=xp_d[0, 1])
            nc.scalar.dma_start(out=xs[(0, 2)], in_=xp_d[0, 2])
            nc.scalar.dma_start(out=xs[(0, 3)], in_=xp_d[0, 3])
            for c in range(NCH):
                nc.gpsimd.dma_start(out=xs[(1, c)], in_=xp_d[1, c])
            for c in range(1, NCH):
                nc.sync.dma_start(out=wt_sb[c], in_=wt_d[c])
            # zero the h-pad rows on DVE: it is idle during the x DMA wait,
            # and on gpsimd these would queue behind 9 DMA descriptor gens,
            # gating the first V transform
            for sm in range(SPC):
                for c in range(NCH):
                    nc.vector.memset(xm[sm][c][:, 0, :], 0.0)
                    nc.vector.memset(xm[sm][c][:, HP - 1, :], 0.0)

            # --- demod matvec: dsum[:,ob,:] = sum_c wsq[c,ob-block]^T (1+s)^2
            # Matmuls are emitted mid-wave (after the first conv group) so
            # the wsq transfer is off conv's critical path; den_s sits at the
            # head of ACT's queue. The M tile recycling dsum's slot has its
            # first write pushed a full group later (see wave()).
            dsum = psum.tile([128, OB, SPC], F32, name="dsum", tag="acc")
            for ob in range(OB):
                for c in range(NCH):
                    nc.tensor.matmul(
                        dsum[:, ob, :],
                        lhsT=wsq_sb[:, c, ob * 128:(ob + 1) * 128],
                        rhs=q[:, c, :],
                        start=(c == 0), stop=(c == NCH - 1),
                    )
            for ob in range(OB):
                nc.scalar.activation(den_s[:, ob, :], dsum[:, ob, :],
                                     mybir.ActivationFunctionType.Sqrt,
                                     bias=eps_t)

            # --- modulation + input transform: the first chunk's mod runs on
            # DVE (lowest latency for wave A's start); the rest on ACT so mod
            # and transform pipeline across engines. V transforms on DVE.
            for sm in range(SPC):
                for c in range(NCH):
                    if sm == 0 and c == 0:
                        nc.vector.tensor_scalar_mul(xm[sm][c][:, 1:H + 1, :],
                                                    xs[(sm, c)],
                                                    s1[:, c, sm:sm + 1])
                    else:
                        nc.scalar.mul(xm[sm][c][:, 1:H + 1, :], xs[(sm, c)],
                                      s1[:, c, sm:sm + 1])
                    xr = xm[sm][c].rearrange("p (a b) w -> p a b w", b=2)
                    ev0, ev1 = xr[:, 0:16, 0, :], xr[:, 1:17, 0, :]
                    od0, od1 = xr[:, 0:16, 1, :], xr[:, 1:17, 1, :]
                    Vc = V[sm][c]
                    nc.vector.tensor_sub(Vc[0], ev0, ev1)
                    nc.vector.tensor_add(Vc[1], od0, ev1)
                    nc.vector.tensor_sub(Vc[2], ev1, od0)
                    nc.vector.tensor_sub(Vc[3], od0, od1)
                if sm == 0:
                    nc.vector.reciprocal(den, den_s)
                    nc.vector.tensor_scalar_mul(dneg, den, -1.0)

            # --- conv waves: per sample, 8 banks = (2 ob x 4 a), each
            # accumulating 12 matmuls (4 c x 3 kx shifted windows). The
            # zero-pad cols of V contribute nothing, so kx=0 skips out col 0
            # and kx=2 skips out col 31 (PSUM has_written covers first hits).
            M = {}

            # V has no w-pad: kx window = [c_lo+kx-1, ...) in V coords
            KXW = {0: (1, 0, 31), 1: (0, 0, 32), 2: (0, 1, 31)}  # out_lo, v_lo, n

            def wave(sm):
                def mm(c, ob, kx, a):
                    o_lo, v_lo, n_c = KXW[kx]
                    key = (sm, ob, a)
                    if key not in M:
                        M[key] = psum.tile([128, 16, W], F32,
                                           name=f"m{sm}_{ob}_{a}", tag="acc")
                    nc.tensor.matmul(
                        M[key][:, :, o_lo:o_lo + n_c],
                        lhsT=wt_sb[c][:, kx, a, ob * 128:(ob + 1) * 128],
                        rhs=V[sm][c][a][:, :, v_lo:v_lo + n_c],
                        start=(c == 0 and kx == 0),
                        stop=(c == NCH - 1 and kx == 2),
                    )

                # kx-outer over both cout blocks: a bank is revisited only
                # every 8 matmuls (4-apart hits a PSUM turnaround gap). The
                # last chunk ends with the kx2 groups per-ob, a-order 1,2,3,0
                # -- the drain chain starts from M1 (ACT d1 copy), so its
                # accumulator should stop first and M0 (needed one op later)
                # last.
                for c in range(NCH):
                    for kx in range(3):
                        for ob in range(OB):
                            if c == NCH - 1 and kx == 2:
                                continue
                            for a in range(4):
                                mm(c, ob, kx, a)
                    if c == NCH - 1:
                        for ob in range(OB):
                            for a in (1, 2, 3, 0):
                                mm(c, ob, 2, a)

            def drain(sm, ob):
                # z0 = (M0+M1+M2)*den -> even rows; z1 = (M1-M2-M3)*den ->
                # odd rows. den folds into the adds via scalar_tensor_tensor
                # (out = (in0*scalar) op in1): ACT pre-scales d1s = M1*den,
                # the z1 path uses -den so no trailing negate/scale is
                # needed. Each op reads at most one PSUM bank. z0/z1 DMA out
                # directly as row-parity planes; the host interleaves rows.
                AL = mybir.AluOpType
                dn = den[:, ob, sm:sm + 1]
                dg = dneg[:, ob, sm:sm + 1]
                # z1-path first: M1/M2/M3 stop before M0 (kx2 a-order 1,2,3,0)
                # so it overlaps the final matmuls; only u0->z0 trails them.
                m = [M[(sm, ob, a)].rearrange("p t w -> p (t w)") for a in range(4)]
                d1s = up.tile([128, 512], F32, name=f"d1_{sm}_{ob}", tag="u", bufs=6)
                nc.scalar.mul(d1s, m[1], dn)
                u1 = up.tile([128, 512], F32, name=f"u1_{sm}_{ob}", tag="u", bufs=6)
                z1 = zp.tile([128, 512], BF16, name=f"z1_{sm}_{ob}", tag="z", bufs=4)
                nc.vector.scalar_tensor_tensor(u1, m[2], dg, d1s, AL.mult, AL.add)
                nc.vector.scalar_tensor_tensor(z1, m[3], dg, u1, AL.mult, AL.add)
                nc.sync.dma_start(out=y_d[sm, ob, 1], in_=z1)
                u0 = up.tile([128, 512], F32, name=f"u0_{sm}_{ob}", tag="u", bufs=6)
                z0 = zp.tile([128, 512], BF16, name=f"z0_{sm}_{ob}", tag="z", bufs=4)
                nc.vector.scalar_tensor_tensor(u0, m[0], dn, d1s, AL.mult, AL.add)
                nc.vector.scalar_tensor_tensor(z0, m[2], dn, u0, AL.mult, AL.add)
                nc.sync.dma_start(out=y_d[sm, ob, 0], in_=z0)

            wave(0)
            drain(0, 0)
            drain(0, 1)
            wave(1)
            drain(1, 0)
            drain(1, 1)

    nc.compile()
    return nc


_G = np.array(
    [[1.0, 0.0, 0.0], [0.5, 0.5, 0.5], [0.5, -0.5, 0.5], [0.0, 0.0, 1.0]],
    np.float32)


def prepare_in_maps(x, s, w):
    """Shard + pack full inputs into per-core in_maps (core = g*QGRID + h)."""
    x = np.asarray(x, dtype=np.float32)
    s = np.asarray(s, dtype=np.float32)
    w = np.asarray(w, dtype=np.float32)

    # h-transformed weights: wt1[kx,a,i,o] = sum_p G[a,p] w[o,i,p,kx]
    wt1 = np.einsum("ap,oipk->kaio", _G, w, optimize=True)
    # -> [cin, kx, a, cout] -> [NCH,128,3,4,COUT]
    wt_l = np.ascontiguousarray(wt1.transpose(2, 0, 1, 3)).reshape(
        NCH, 128, 3, 4, COUT).astype(BF)
    wsq = np.sum(w * w, axis=(2, 3)).T  # [cin, cout]
    wsq_l = np.ascontiguousarray(
        wsq.reshape(NCH, 128, COUT).transpose(1, 0, 2)).astype(BF)

    xp_all = x.astype(BF).reshape(PGRID, SPC, NCH, 128, H, W)
    s1_all = (s + 1.0).reshape(PGRID, SPC, NCH, 128).transpose(0, 3, 2, 1)
    q_all = (s1_all * s1_all).astype(BF)

    in_maps = []
    for g in range(PGRID):
        for h in range(QGRID):
            in_maps.append({
                "xp": np.ascontiguousarray(xp_all[g]),
                "s1p": np.ascontiguousarray(s1_all[g]),
                "qp": np.ascontiguousarray(q_all[g]),
                "wt": np.ascontiguousarray(wt_l[:, :, :, :, h * OHALF:(h + 1) * OHALF]),
                "wsq": np.ascontiguousarray(wsq_l[:, :, h * OHALF:(h + 1) * OHALF]),
            })
    return in_maps


def assemble_output(results):
    y = np.zeros((B, COUT, H, W), np.float32)
    for g in range(PGRID):
        for h in range(QGRID):
            r = results[g * QGRID + h]["y"].astype(np.float32)
            for sm in range(SPC):
                for ob in range(OB):
                    blk = y[g * SPC + sm,
                            h * OHALF + ob * 128:h * OHALF + (ob + 1) * 128]
                    blk[:, 0::2, :] = r[sm, ob, 0].reshape(128, 16, W)
                    blk[:, 1::2, :] = r[sm, ob, 1].reshape(128, 16, W)
    return y


def kernel(x, s, w):
    from concourse.bass_utils import run_bass_kernel_spmd

    global _compiled_nc
    if _compiled_nc is None:
        _compiled_nc = _build()
    nc = _compiled_nc

    in_maps = prepare_in_maps(x, s, w)
    res = run_bass_kernel_spmd(nc, in_maps, list(range(B))).results
    return assemble_output(res)
